# revision 31
# baseline (speedup 1.0000x reference)
"""Trainium2 Bass kernel: gradient of the EnergyAttention scalar energy.

reference:
    q = einsum('bqd,hzd->bqhz', g, wq); k = einsum('bkd,hzd->bkhz', g, wk)
    scores = einsum('bqhz,bkhz->bhqk', q, k)
    E = -(logsumexp(BETA*scores, -1)/BETA).sum() + POS_SCALE*(g*pos).sum()
    out = dE/dg

Math: with E = exp(BETA*scores), Z = E.1, per (b,h):
    out[b] = -sum_h [ diag(1/Z) E K wq_h + E^T diag(1/Z) Q wk_h ] + POS_SCALE*pos

Sharding: 8 cores; core c handles batch b=c//4 and heads 4*(c%4)..4*(c%4)+3
(two head-pairs packed into the 128-partition dim).  Each core returns its
full [S, D] positive partial in fp16; the host sums 4 partials per batch and
applies the positional term (no on-device collectives -- slow under this
runtime).

Design, engineered against on-device NTFF profiles (prior baseline
182 us measured the same way; this version ~108 us):
  * x and wq/wk are uploaded PRE-TRANSPOSED from the host (same byte count)
    -- no PE transposes for x^T / W^T.  Qraw/K2n [s, z2] come from XBAR DMA
    transposes (idle DMA engines, ~1 us each).
  * scores blocks are contraction-64 matmuls issued strictly alternating
    head a/b: consecutive instructions land in disjoint PE row groups
    (tile_position (0,0)/(64,0)) and execute CONCURRENTLY (measured ~2x).
    dK/dQ are M=64 matmuls, col-group paired the same way ((0,0)/(0,64)).
  * ONE exp pass per scores block (ACT) with fused row-sum accumulation for
    Z; the transposed E^T tiles for the dQ path come from XBAR DMA
    transposes of the fp16 E tiles, NOT a scoresT recompute + second exp
    pass (which would double ACT time, the #2 engine).
  * Software pipelining keeps every engine fed and avoids PE FIFO
    head-of-line stalls: warm-up matmuls hold the PE clock at 2.4 GHz
    through the load phase; dK_i runs one iteration behind exp_i; pair-1's
    projections fill pair-0's loop; pair-0's deferred dQ burst covers the
    pair transition; pair-0's output-projection terms run inside pair-1's
    loop; the tail interleaves pair-1's dQ with the final output pass and
    the gout DMA is chunked per q-block.
  * The 1/Z rows for the dQ rescale are built entirely on-core (recip ->
    f16 cast -> PE transpose -> select+broadcast matmuls) -- the earlier
    DMA + gpsimd partition_broadcast chain sat behind the XBAR descriptor
    backlog and stalled ACT ~10 us at each pair transition.
"""

import numpy as np

B = 2
S = 1024
D = 1024
NH = 16
Z = 64
BETA = 1.0 / np.sqrt(np.float32(Z))
POS_SCALE = 0.001
N_CORES = 8
HPC = 4           # heads per core
NPAIR = 2         # head pairs per core
ND = D // 128     # 8 d-tiles
NQ = S // 128     # 8 q/k blocks
NCH = S // 512    # 2 moving-dim chunks

_CACHE = {}


def build_nc(reps=1):
    """Build the (SPMD, identical-per-core) Bass program.

    reps>1 repeats the whole computation (idempotent) inside one NEFF --
    used for marginal-cost timing."""
    from contextlib import ExitStack

    import concourse.mybir as mybir
    import concourse.tile as tile
    from concourse import bacc
    from concourse.masks import make_identity

    F32 = mybir.dt.float32
    F16 = mybir.dt.float16
    MUL = mybir.AluOpType.mult
    ADD = mybir.AluOpType.add
    EXP = mybir.ActivationFunctionType.Exp

    nc = bacc.Bacc(
        "TRN2",
        target_bir_lowering=False,
        debug=False,
        enable_asserts=False,
        num_devices=N_CORES,
    )

    # Pre-transposed x:  xt[d, s] = x[b][s, d]
    xt_in = nc.dram_tensor("xt", [D, S], F16, kind="ExternalInput").ap()
    # Natural weights [(qk, pair, z2), d] for the output projection
    wn_in = nc.dram_tensor("wn", [2 * NPAIR * 128, D], F16, kind="ExternalInput").ap()
    # Pre-transposed weights [d % 128, (qk, pair, dt, z2)] for the projections
    wt_in = nc.dram_tensor("wt", [128, 2 * NPAIR * ND * 128], F16,
                           kind="ExternalInput").ap()
    gout = nc.dram_tensor("gout", [S, D], F16, kind="ExternalOutput").ap()

    with tile.TileContext(nc) as tc, ExitStack() as ctx:
        sb1 = ctx.enter_context(tc.tile_pool(name="sb1", bufs=1))
        sb2 = ctx.enter_context(tc.tile_pool(name="sb2", bufs=2))
        sb4 = ctx.enter_context(tc.tile_pool(name="sb4", bufs=4))
        pp = ctx.enter_context(tc.tile_pool(name="pp", bufs=1))
        # PSUM (8 banks of [128, 2KB]): "sc" rotates scores/out tiles
        # ([128,1024]f32 = 2 banks, bufs=2 -> 4 banks); "d" rotates the
        # dK/dQ accumulators and the interleaved projection tiles (4 banks).
        ps_sc = ctx.enter_context(tc.tile_pool(name="ps_sc", bufs=2, space="PSUM"))
        ps_d = ctx.enter_context(tc.tile_pool(name="ps_d", bufs=2, space="PSUM"))

        ident = sb1.tile([128, 128], F32, tag="ident")
        make_identity(nc, ident[:])

        for _rep in range(reps):
            # ---- loads (3 merged DMAs) -----------------------------------
            gt = sb1.tile([128, ND * S], F16, tag="gt")   # [d%128, (dt, s)]
            gt_r = gt[:].rearrange("p (dt s) -> p dt s", dt=ND)
            xt_r = xt_in[:].rearrange("(dt p) s -> p dt s", p=128)
            for q4 in range(4):
                nc.sync.dma_start(gt_r[:, 2 * q4 : 2 * q4 + 2, :],
                                  xt_r[:, 2 * q4 : 2 * q4 + 2, :])
            wt_all = sb1.tile([128, 2 * NPAIR * ND * 128], F16, tag="wt_all")
            wh = NPAIR * ND * 128
            nc.scalar.dma_start(wt_all[:, 0:wh], wt_in[:, 0:wh])
            nc.scalar.dma_start(wt_all[:, wh : 2 * wh], wt_in[:, wh : 2 * wh])
            wn_all = sb1.tile([128, 2 * NPAIR * D], F16, tag="wn_all")
            nc.scalar.dma_start(
                wn_all[:].rearrange("p (b d) -> p b d", b=2 * NPAIR),
                wn_in[:].rearrange("(b p) d -> p b d", p=128),
            )

            # persistent across pairs
            dqt2 = sb1.tile([128, NPAIR * S], F16, tag="dqt2")  # [z2, (pair, q)]
            dkt2 = sb1.tile([128, NPAIR * S], F16, tag="dkt2")  # [z2, (pair, k)]
            go_all = sb1.tile([128, NQ * S], F16, tag="go_all")
            selmat = sb1.tile([16, 16 * 128], F16, tag="selmat")
            nc.gpsimd.memset(selmat[:], 0.0)
            nc.gpsimd.affine_select(
                out=selmat[:].rearrange("p (sl m) -> p sl m", sl=16),
                in_=selmat[:].rearrange("p (sl m) -> p sl m", sl=16),
                compare_op=mybir.AluOpType.not_equal,
                fill=1.0,
                base=0,
                pattern=[[-1, 16], [0, 128]],
                channel_multiplier=1,
            )
            ztsb = sb1.tile([16, 128], F16, tag="ztsb")

            ident_h = sb1.tile([128, 512], F16, tag="ident_h")
            for c in range(4):
                nc.vector.tensor_copy(ident_h[:, c * 128 : (c + 1) * 128], ident[:])
            for w in range(30):
                wps = ps_sc.tile([128, 512], F32, tag="ps_sc", name=f"warm{_rep}_{w}")
                nc.tensor.matmul(wps[:], lhsT=ident_h[:, 0:128], rhs=ident_h[:],
                                 start=True, stop=True)
                if w == 29:
                    nc.vector.tensor_copy(ztsb[:], wps[0:16, 0:128])

            def wt_blk(qk, p, dt):
                j = (qk * NPAIR + p) * ND + dt
                return wt_all[:, j * 128 : (j + 1) * 128]

            pairs = []  # per-pair tiles

            def proj_chunk(p, qk, ps, dts, dst):
                """Two d-tiles of the Q/K projection for pair p."""
                for dt in dts:
                    for ch in range(NCH):
                        nc.tensor.matmul(
                            ps[:, ch * 512 : (ch + 1) * 512],
                            lhsT=wt_blk(qk, p, dt),
                            rhs=gt[:, dt * S + ch * 512 : dt * S + ch * 512 + 512],
                            start=(dt == 0),
                            stop=(dt == ND - 1),
                        )
                if dst is not None and dts[-1] == ND - 1:
                    nc.vector.tensor_copy(dst[:], ps[:])

            def scores_exp(p, i):
                """Scores blocks [q_i, k] (row-group paired) + one exp/head."""
                qt2, kt2 = pairs[p]["qt2"], pairs[p]["kt2"]
                pt_s = [ps_sc.tile([128, S], F32, tag="ps_sc",
                                   name=f"sc{_rep}_{p}_{i}_{a}")
                        for a in range(2)]
                for ch, a in ((0, 0), (0, 1), (1, 1), (1, 0)):
                    nc.tensor.matmul(
                        pt_s[a][:, ch * 512 : (ch + 1) * 512],
                        lhsT=qt2[a * 64 : (a + 1) * 64, i * 128 : (i + 1) * 128],
                        rhs=kt2[a * 64 : (a + 1) * 64, ch * 512 : (ch + 1) * 512],
                        start=True,
                        stop=True,
                        tile_position=(a * 64, 0),
                    )
                P_all, PT_r = pairs[p]["P_all"], pairs[p]["PT_r"]
                zsum2 = pairs[p]["zsum2"]
                for a in range(2):
                    nc.scalar.activation(
                        P_all[:, (a * NQ + i) * S : (a * NQ + i + 1) * S],
                        pt_s[a][:],
                        EXP,
                        scale=float(BETA),
                        accum_out=zsum2[:, a * NQ + i : a * NQ + i + 1],
                    )
                    nc.sync.dma_start_transpose(
                        PT_r[:, a, :, i * 128 : (i + 1) * 128],
                        P_all[:, (a * NQ + i) * S : (a * NQ + i + 1) * S],
                    )

            def dk_step(p, j):
                """q2n_j then dK^T += Qn_j^T E_j (col-group paired); emitted
                one iteration behind exp_j so the PE never waits on ACT."""
                qraw, zsum2 = pairs[p]["qraw"], pairs[p]["zsum2"]
                P_all, dk_ps = pairs[p]["P_all"], pairs[p]["dk_ps"]
                q2n_t = sb4.tile([128, 128], F16, tag="q2n",
                                 name=f"q2n{_rep}_{p}_{j}")
                for a in range(2):
                    zq = sb4.tile([128, 1], F32, tag="zq",
                                  name=f"zq{_rep}_{p}_{j}_{a}")
                    nc.vector.reciprocal(
                        zq[:], zsum2[:, a * NQ + j : a * NQ + j + 1])
                    nc.vector.tensor_scalar_mul(
                        q2n_t[:, a * 64 : (a + 1) * 64],
                        qraw[:, j * 128 + a * 64 : j * 128 + (a + 1) * 64],
                        zq[:],
                    )
                for ch, a in ((0, 0), (0, 1), (1, 1), (1, 0)):
                    nc.tensor.matmul(
                        dk_ps[a * 64 : (a + 1) * 64, ch * 512 : (ch + 1) * 512],
                        lhsT=q2n_t[:, a * 64 : (a + 1) * 64],
                        rhs=P_all[:, (a * NQ + j) * S + ch * 512 :
                                  (a * NQ + j) * S + ch * 512 + 512],
                        start=(j == 0),
                        stop=(j == NQ - 1),
                        tile_position=(0, a * 64),
                        skip_group_check=True,
                    )

            def emit_dq_burst(p, tag):
                """Deferred dQ^T(unnorm): 8 col-group-paired accumulation
                steps over k-blocks, then Z-rescale into dqt2."""
                dq_ps = ps_d.tile([128, S], F32, tag="ps_d", name=f"dqp{_rep}_{tag}")
                k2n_p, PT_rp = pairs[p]["k2n"], pairs[p]["PT_r"]
                for ch in range(NCH):
                    for i in range(NQ):
                        for a in ((0, 1) if i % 2 == 0 else (1, 0)):
                            nc.tensor.matmul(
                                dq_ps[a * 64 : (a + 1) * 64,
                                      ch * 512 : (ch + 1) * 512],
                                lhsT=k2n_p[:, i * 128 + a * 64 : i * 128 + (a + 1) * 64],
                                rhs=PT_rp[:, a, i, ch * 512 : (ch + 1) * 512],
                                start=(i == 0),
                                stop=(i == NQ - 1),
                                tile_position=(0, a * 64),
                                skip_group_check=True,
                            )
                for a in range(2):
                    zbc = pairs[p]["zbcs"][a]
                    nc.vector.tensor_tensor(
                        dqt2[a * 64 : (a + 1) * 64, p * S : (p + 1) * S],
                        dq_ps[a * 64 : (a + 1) * 64, :],
                        zbc[a * 64 : (a + 1) * 64, :],
                        MUL,
                    )

            def out_chain(sb, terms, acc, pool=None, ptag=None):
                """One q-block of the output projection: sum_t dmat_t wn_t."""
                pool = pool if pool is not None else ps_sc
                ptag = ptag or "ps_sc"
                ps = pool.tile([128, S], F32, tag=ptag,
                               name=f"op{_rep}_{sb}_{acc}")
                for ch in range(NCH):
                    tl = terms if ch % 2 == 0 else terms[::-1]
                    for pos, (dmat, qk, pa) in enumerate(tl):
                        nc.tensor.matmul(
                            ps[:, ch * 512 : (ch + 1) * 512],
                            lhsT=dmat[:, pa * S + sb * 128 : pa * S + (sb + 1) * 128],
                            rhs=wn_all[:, (qk * NPAIR + pa) * D + ch * 512 :
                                       (qk * NPAIR + pa) * D + ch * 512 + 512],
                            start=(pos == 0),
                            stop=(pos == len(tl) - 1),
                        )
                if acc:
                    nc.vector.tensor_tensor(
                        go_all[:, sb * S : (sb + 1) * S],
                        go_all[:, sb * S : (sb + 1) * S], ps[:], ADD)
                else:
                    nc.vector.tensor_copy(go_all[:, sb * S : (sb + 1) * S], ps[:])

            def pair_end(p):
                """dK evacuation."""
                dk_ps = pairs[p]["dk_ps"]
                nc.vector.tensor_copy(dkt2[:, p * S : (p + 1) * S], dk_ps[:])

            def zbc_build(p):
                """1/Z broadcast rows [z2, q] for the dQ rescale, built
                entirely on-core: recip -> f16 cast -> PE transpose -> PE
                select+broadcast matmuls -> SBUF (no DMA, no gpsimd)."""
                zsum2 = pairs[p]["zsum2"]
                zinv2 = sb2.tile([128, 16], F32, tag="zinv2")
                nc.vector.reciprocal(zinv2[:], zsum2[:])
                zinv16 = sb2.tile([128, 16], F16, tag="zinv16")
                nc.vector.tensor_copy(zinv16[:], zinv2[:])
                zt_ps = ps_sc.tile([128, 128], F16, tag="ps_sc",
                                   name=f"ztp{_rep}_{p}")
                nc.tensor.transpose(zt_ps[0:16, 0:128], zinv16[:],
                                    ident_h[:, 0:128])
                nc.vector.tensor_copy(ztsb[:], zt_ps[0:16, 0:128])
                zbcs = []
                for a in range(2):
                    zb_ps = ps_sc.tile([128, S], F32, tag="ps_sc",
                                       name=f"zbp{_rep}_{p}_{a}")
                    for i in range(NQ):
                        sl = a * NQ + i
                        nc.tensor.matmul(
                            zb_ps[:, i * 128 : (i + 1) * 128],
                            lhsT=selmat[:, sl * 128 : (sl + 1) * 128],
                            rhs=ztsb[:],
                            start=True,
                            stop=True,
                            tile_position=(0, 0),
                            skip_group_check=True,
                        )
                    zbc = sb2.tile([128, S], F16, tag="zbc",
                                   name=f"zbc{_rep}_{p}_{a}", bufs=2)
                    nc.vector.tensor_copy(zbc[:], zb_ps[:])
                    zbcs.append(zbc)
                pairs[p]["zbcs"] = zbcs

            def alloc_pair(p):
                d = {}
                d["qt2"] = sb2.tile([128, S], F16, tag="qt2", name=f"qt2_{_rep}_{p}")
                d["kt2"] = sb2.tile([128, S], F16, tag="kt2", name=f"kt2_{_rep}_{p}")
                d["qraw"] = sb2.tile([128, NQ * 128], F16, tag="qraw",
                                     name=f"qraw_{_rep}_{p}")
                d["k2n"] = sb2.tile([128, NQ * 128], F16, tag="k2n",
                                    name=f"k2n_{_rep}_{p}")
                d["zsum2"] = sb2.tile([128, 16], F32, tag="zsum2",
                                      name=f"zsum2_{_rep}_{p}")
                d["P_all"] = pp.tile([128, 2 * NQ * S], F16, tag="P",
                                     name=f"P{_rep}_{p}", bufs=2)
                d["PT_all"] = pp.tile([128, 2 * NQ * S], F16, tag="PT",
                                      name=f"PT{_rep}_{p}")
                d["PT_r"] = d["PT_all"][:].rearrange("p (a j s) -> p a j s",
                                                     a=2, j=NQ)
                return d

            def qraw_k2n(p):
                nc.sync.dma_start_transpose(
                    pairs[p]["qraw"][:].rearrange("p (j z) -> p j z", j=NQ),
                    pairs[p]["qt2"][:])
                nc.sync.dma_start_transpose(
                    pairs[p]["k2n"][:].rearrange("p (j z) -> p j z", j=NQ),
                    pairs[p]["kt2"][:])

            # ================= schedule =================
            pairs.append(alloc_pair(0))
            pairs.append(alloc_pair(1))

            # pair-0 projections up front
            for qk in range(2):
                ps = ps_d.tile([128, S], F32, tag="ps_d", name=f"pj{_rep}_0_{qk}")
                proj_chunk(0, qk, ps, list(range(ND)),
                           pairs[0]["qt2"] if qk == 0 else pairs[0]["kt2"])
            qraw_k2n(0)

            # pair-0 loop; pair-1 projections ride along (4 MMs per slot)
            pj_ps = {}
            dk0 = pairs[0]["dk_ps"] = ps_d.tile([128, S], F32, tag="ps_d",
                                                name=f"dk{_rep}_0")
            for i in range(NQ + 1):
                if i < NQ:
                    scores_exp(0, i)
                if i < 4:
                    if i == 0:
                        pj_ps[0] = ps_d.tile([128, S], F32, tag="ps_d",
                                             name=f"pj{_rep}_1_0")
                    proj_chunk(1, 0, pj_ps[0], [2 * i, 2 * i + 1],
                               pairs[1]["qt2"])
                elif i < NQ:
                    if i == 4:
                        pj_ps[1] = ps_d.tile([128, S], F32, tag="ps_d",
                                             name=f"pj{_rep}_1_1")
                    proj_chunk(1, 1, pj_ps[1], [2 * (i - 4), 2 * (i - 4) + 1],
                               pairs[1]["kt2"])
                if i >= 1:
                    dk_step(0, i - 1)
            pair_end(0)
            qraw_k2n(1)

            # pair-1 loop; pair-0's dQ burst covers the transition, pair-0's
            # output-projection terms ride along
            pairs[1]["dk_ps"] = ps_d.tile([128, S], F32, tag="ps_d",
                                          name=f"dk{_rep}_1")
            for i in range(NQ + 1):
                if i < NQ:
                    if i <= 1:
                        with tc.high_priority():
                            scores_exp(1, i)
                    else:
                        scores_exp(1, i)
                if i == 0:
                    zbc_build(0)
                    emit_dq_burst(0, "b1")
                if i >= 1:
                    dk_step(1, i - 1)
                if i >= 1 and i - 1 < NQ - 2:
                    out_chain(i - 1, [(dqt2, 0, 0), (dkt2, 1, 0)], acc=False,
                              pool=ps_d, ptag="ps_d")
            pair_end(1)
            for sb in (NQ - 2, NQ - 1):
                out_chain(sb, [(dqt2, 0, 0), (dkt2, 1, 0)], acc=False,
                          pool=ps_d, ptag="ps_d")
            zbc_build(1)

            # tail: pair-1 dQ + the pair-1 output-projection terms.
            # dk-half chains first (ready at pair_end), dq-half after the
            # rescale; evacuations split DVE/GpSimd; gout DMA chunked.
            emit_dq_burst(1, "tail")
            for sb in range(NQ):
                if sb % 2 == 0:
                    out_chain(sb, [(dkt2, 1, 1), (dqt2, 0, 1)], acc=True)
                else:
                    out_chain(sb, [(dkt2, 1, 1), (dqt2, 0, 1)], acc=True,
                              pool=ps_d, ptag="ps_d")
                nc.sync.dma_start(
                    gout[sb * 128 : (sb + 1) * 128, :],
                    go_all[:, sb * S : (sb + 1) * S],
                )

    nc.compile()
    return nc


def core_inputs(x, wq, wk, core):
    """Per-core input arrays (host-side shard/layout prep, all cheap)."""
    b = core // 4
    h0 = 4 * (core % 4)
    xt = np.ascontiguousarray(x[b].T).astype(np.float16)
    wq4 = wq[h0 : h0 + 4].reshape(NPAIR, 128, D)
    wk4 = wk[h0 : h0 + 4].reshape(NPAIR, 128, D)
    wn = np.concatenate(
        [wq4.reshape(NPAIR * 128, D), wk4.reshape(NPAIR * 128, D)]
    ).astype(np.float16)
    # wt[p, (qk, pair, dt, z2)] = w[qk][pair, z2, dt*128 + p]
    wstack = np.stack([wq4, wk4])                    # [qk, pair, z2, d]
    wt = (
        wstack.reshape(2, NPAIR, 128, ND, 128)       # [qk, pair, z2, dt, p]
        .transpose(4, 0, 1, 3, 2)                    # [p, qk, pair, dt, z2]
        .reshape(128, 2 * NPAIR * ND * 128)
    )
    wt = np.ascontiguousarray(wt).astype(np.float16)
    return {"xt": xt, "wn": wn, "wt": wt}


def combine(gouts):
    """Host unshard: sum the 4 positive partials per batch, apply pos term."""
    pos = np.linspace(-0.5, 0.5, S, dtype=np.float32)[:, None] * np.float32(POS_SCALE)
    out = np.empty((B, S, D), np.float32)
    for b in range(B):
        acc = np.asarray(gouts[4 * b], np.float32)
        for c in range(4 * b + 1, 4 * b + 4):
            acc += np.asarray(gouts[c], np.float32)
        out[b] = pos - acc
    return out


def _build_persistent(nc):
    """One-time jitted sharded callable over the Bass NEFF (no per-call
    retracing; outputs are fully written by the kernel so no donation)."""
    import jax
    import numpy as _np
    from jax.experimental.shard_map import shard_map
    from jax.sharding import Mesh, NamedSharding, PartitionSpec

    import concourse.mybir as mybir
    from concourse import bass2jax

    bass2jax.install_neuronx_cc_hook()
    partition_name = nc.partition_id_tensor.name if nc.partition_id_tensor else None
    in_names, out_names, out_avals = [], [], []
    for alloc in nc.m.functions[0].allocations:
        if not isinstance(alloc, mybir.MemoryLocationSet):
            continue
        name = alloc.memorylocations[0].name
        if alloc.kind == "ExternalInput":
            if name != partition_name:
                in_names.append(name)
        elif alloc.kind == "ExternalOutput":
            out_names.append(name)
            out_avals.append(
                jax.core.ShapedArray(tuple(alloc.tensor_shape), mybir.dt.np(alloc.dtype))
            )
    n_params = len(in_names)
    all_in_names = list(in_names) + out_names
    if partition_name is not None:
        all_in_names.append(partition_name)

    def _body(*args):
        operands = list(args)
        if partition_name is not None:
            operands.append(bass2jax.partition_id_tensor())
        return tuple(
            bass2jax._bass_exec_p.bind(
                *operands,
                out_avals=tuple(out_avals),
                in_names=tuple(all_in_names),
                out_names=tuple(out_names),
                lowering_input_output_aliases=(),
                sim_require_finite=True,
                sim_require_nnan=True,
                nc=nc,
            )
        )

    devices = jax.devices()[:N_CORES]
    mesh = Mesh(_np.asarray(devices), ("core",))
    spec = PartitionSpec("core")
    sharded = jax.jit(
        shard_map(
            _body,
            mesh=mesh,
            in_specs=(spec,) * (n_params + len(out_names)),
            out_specs=(spec,) * len(out_names),
            check_rep=False,
        ),
        keep_unused=True,
    )
    sh = NamedSharding(mesh, spec)
    zeros = [
        jax.device_put(
            _np.zeros((N_CORES * a.shape[0],) + a.shape[1:], a.dtype), sh
        )
        for a in out_avals
    ]
    return {
        "sharded": sharded,
        "in_names": in_names,
        "out_names": out_names,
        "out_avals": out_avals,
        "sh": sh,
        "zeros": zeros,
        "jax": jax,
    }


def kernel(x, wq, wk):
    x = np.asarray(x, np.float32)
    wq = np.asarray(wq, np.float32)
    wk = np.asarray(wk, np.float32)
    if "nc" not in _CACHE:
        _CACHE["nc"] = build_nc()
    nc = _CACHE["nc"]
    if "pc" not in _CACHE:
        _CACHE["pc"] = _build_persistent(nc)
    pc = _CACHE["pc"]
    jax = pc["jax"]

    in_maps = [core_inputs(x, wq, wk, c) for c in range(N_CORES)]
    concat_in = [
        jax.device_put(
            np.concatenate([np.asarray(m[nm]) for m in in_maps], axis=0), pc["sh"]
        )
        for nm in pc["in_names"]
    ]
    outs = pc["sharded"](*concat_in, *pc["zeros"])
    g = np.asarray(outs[pc["out_names"].index("gout")])
    return combine(g.reshape(N_CORES, S, D))
